# revision 2
# baseline (speedup 1.0000x reference)
"""iSTFT synthesis head (1x1 conv -> exp/cos/sin -> irfft -> windowed overlap-add)
as a Bass/Tile kernel for 8 Trainium2 NeuronCores, data-parallel over batch.

Math layout (validated in numpy to 1e-6 vs the jax reference, bf16 device
precision ~4e-3 relative):
  y = Wp @ x + bp  (rows permuted: [mag_0..511, phase_0..511, mag_512, phase_512])
  amp = exp(min(mag, 4));  v = [amp*cos(ph) (k=0..511), real_512, amp*sin(ph) (k=1..511)]
  frames_w[n, t] = sum_kv coef[kv, n] * v[kv, t]   (window+irfft folded into coef)
  overlap-add is folded into the same matmul: with n = 256*j + r,
  out_T[s, r] = sum_j sum_kv coef[kv, 256j+r] * v[kv, s-j]
  final[S, 0:128] = out_T[S+1, 128:256] / env, final[S, 128:256] = out_T[S+2, 0:128] / env
"""

import math

import numpy as np
import ml_dtypes

B, C, T = 8, 512, 2048
NFFT, HOP = 1024, 256
NSLOT = 2051
OUTLEN = 524288
MAG_CLAMP = 4.0
N_CORES = 8

_BF16 = ml_dtypes.bfloat16

_compiled = None  # (nc,) cache — program depends only on shapes


def _build_host_constants(W, b, window):
    """Host-side constant tensors (fp64 math, cast at the end)."""
    w = np.asarray(window, np.float64)
    Wf = np.asarray(W, np.float64)
    bf = np.asarray(b, np.float64)

    n = np.arange(NFFT)
    coef = np.zeros((NFFT, NFFT))
    kv = np.arange(512)
    ck = np.where(kv == 0, 1.0, 2.0)
    coef[:512, :] = (ck[:, None] / NFFT) * np.cos(2 * np.pi * kv[:, None] * n[None, :] / NFFT)
    coef[512, :] = (1.0 / NFFT) * np.cos(np.pi * n)
    ki = np.arange(513, 1024) - 512
    coef[513:, :] = -(2.0 / NFFT) * np.sin(2 * np.pi * ki[:, None] * n[None, :] / NFFT)
    coef *= w[None, :]

    BAS = np.zeros((128, 32, 256))
    for j in range(4):
        for i in range(8):
            BAS[:, 8 * j + i, :] = coef[i * 128:(i + 1) * 128, 256 * j:256 * j + 256]

    win2 = w * w
    env2d = np.zeros((NSLOT, 256))
    for j in range(4):
        env2d[j:j + T, :] += win2[256 * j:256 * j + 256][None, :]
    env2d = np.where(env2d > 1e-11, env2d, 1.0)
    envinv2d = 1.0 / env2d
    ENVINV = np.zeros((128, 17, 256))
    for ci in range(16):
        ENVINV[:, ci, :] = envinv2d[128 * ci:128 * ci + 128, :]
    ENVINV[:, 16, :] = envinv2d[1921:1921 + 128, :]

    # permuted weights, transposed to (K=512 partition-blocks, M=1026)
    perm = np.concatenate([np.arange(512), 513 + np.arange(512), [512], [1025]])
    Wp = Wf[perm]            # (1026, 512)
    bp = bf[perm]            # (1026,)
    WT = Wp.T.reshape(4, 128, 1026).transpose(1, 0, 2)   # (128, 4, 1026)

    # bias vectors: cols 0..3 = b_mag blocks, 4..7 = b_phase + pi/2, 8..11 = b_phase
    BVEC = np.zeros((128, 12))
    for j in range(4):
        BVEC[:, j] = bp[128 * j:128 * j + 128]
        BVEC[:, 4 + j] = bp[512 + 128 * j:512 + 128 * j + 128] + math.pi / 2
        BVEC[:, 8 + j] = bp[512 + 128 * j:512 + 128 * j + 128]
    BSPEC = np.zeros((2, 2))
    BSPEC[0, 0] = bp[1024]                    # b_mag512 for the clamp-add
    BSPEC[1, 1] = bp[1025] + math.pi / 2      # b_phase512 + pi/2 for cos row

    return {
        "wt": np.ascontiguousarray(WT.astype(_BF16)),
        "bas": np.ascontiguousarray(BAS.astype(_BF16)),
        "envinv": np.ascontiguousarray(ENVINV.astype(np.float32)),
        "bvec": np.ascontiguousarray(BVEC.astype(np.float32)),
        "bspec": np.ascontiguousarray(BSPEC.astype(np.float32)),
    }


def _build_program():
    from contextlib import ExitStack

    import concourse.bacc as bacc
    import concourse.tile as tile
    import concourse.mybir as mybir

    f32 = mybir.dt.float32
    bf16 = mybir.dt.bfloat16
    Exp = mybir.ActivationFunctionType.Exp
    Sin = mybir.ActivationFunctionType.Sin
    ADD = mybir.AluOpType.add
    MIN = mybir.AluOpType.min

    nc = bacc.Bacc("TRN2", target_bir_lowering=False, debug=False,
                   num_devices=N_CORES)

    x_d = nc.dram_tensor("x", [128, 4, T], bf16, kind="ExternalInput")
    wt_d = nc.dram_tensor("wt", [128, 4, 1026], bf16, kind="ExternalInput")
    bas_d = nc.dram_tensor("bas", [128, 32, 256], bf16, kind="ExternalInput")
    env_d = nc.dram_tensor("envinv", [128, 17, 256], f32, kind="ExternalInput")
    bvec_d = nc.dram_tensor("bvec", [128, 12], f32, kind="ExternalInput")
    bspec_d = nc.dram_tensor("bspec", [2, 2], f32, kind="ExternalInput")
    out_d = nc.dram_tensor("out", [2048, 256], f32, kind="ExternalOutput")

    with tile.TileContext(nc) as tc, ExitStack() as ctx:
        consts = ctx.enter_context(tc.tile_pool(name="consts", bufs=1))
        ypsum = ctx.enter_context(tc.tile_pool(name="ypsum", bufs=3, space="PSUM"))
        spsum = ctx.enter_context(tc.tile_pool(name="spsum", bufs=2, space="PSUM"))
        opsum = ctx.enter_context(tc.tile_pool(name="opsum", bufs=2, space="PSUM"))
        tpool = ctx.enter_context(tc.tile_pool(name="tpool", bufs=4))
        vpool = ctx.enter_context(tc.tile_pool(name="vpool", bufs=2))
        opool = ctx.enter_context(tc.tile_pool(name="opool", bufs=3))
        spool = ctx.enter_context(tc.tile_pool(name="spool", bufs=2))

        wt_sb = consts.tile([128, 4, 1026], bf16)
        bas_sb = consts.tile([128, 32, 256], bf16)
        env_sb = consts.tile([128, 17, 256], f32)
        bvec_sb = consts.tile([128, 12], f32)
        bspec_sb = consts.tile([2, 2], f32)
        x_sb = consts.tile([128, 4, T], bf16)
        amp_sb = consts.tile([128, 4, T], f32)
        amp512_sb = consts.tile([1, T], f32)
        strip2_sb = consts.tile([2, T], f32)
        ztile = consts.tile([1, 128], f32)

        nc.sync.dma_start(out=wt_sb, in_=wt_d[:, :, :])
        nc.sync.dma_start(out=bas_sb, in_=bas_d[:, :, :])
        nc.sync.dma_start(out=env_sb, in_=env_d[:, :, :])
        nc.sync.dma_start(out=bvec_sb, in_=bvec_d[:, :])
        nc.sync.dma_start(out=bspec_sb, in_=bspec_d[:, :])
        for c in range(4):
            sl = slice(512 * c, 512 * c + 512)
            nc.sync.dma_start(out=x_sb[:, :, sl], in_=x_d[:, :, sl])
        nc.vector.memset(ztile, 0.0)

        # ---- Pass A: magnitudes -> amp (all Exp together: one ACT table set)
        for c in range(4):
            sl = slice(512 * c, 512 * c + 512)
            for j in range(4):
                yp = ypsum.tile([128, 512], f32, tag="yp")
                for k in range(4):
                    nc.tensor.matmul(
                        yp[:, :], wt_sb[:, k, 128 * j:128 * j + 128], x_sb[:, k, sl],
                        start=(k == 0), stop=(k == 3))
                t = tpool.tile([128, 512], f32, tag="t")
                nc.vector.tensor_scalar(
                    t[:, :], yp[:, :], bvec_sb[:, j:j + 1], 4.0, ADD, MIN)
                nc.scalar.activation(amp_sb[:, j, sl], t[:, :], Exp)
            # special 2-row block: [mag_512, phase_512]
            sp = spsum.tile([2, 512], f32, tag="sp")
            for k in range(4):
                nc.tensor.matmul(
                    sp[:, :], wt_sb[:, k, 1024:1026], x_sb[:, k, sl],
                    start=(k == 0), stop=(k == 3))
            nc.vector.tensor_copy(out=strip2_sb[:, sl], in_=sp[:, :])
            t2 = spool.tile([2, 512], f32, tag="t2")
            nc.vector.tensor_scalar(
                t2[:, :], sp[:, :], bspec_sb[:, 0:1], 4.0, ADD, MIN)
            a2 = spool.tile([2, 512], f32, tag="a2")
            nc.scalar.activation(a2[:, :], t2[:, :], Exp)
            nc.vector.tensor_copy(out=amp512_sb[0:1, sl], in_=a2[0:1, :])

        # ---- Pass B: phases -> v -> fused irfft+OLA matmul -> env -> out
        vprev = None
        for c in range(4):
            sl = slice(512 * c, 512 * c + 512)
            v = vpool.tile([128, 8, 516], bf16, tag="v")
            if c == 0:
                nc.vector.memset(v[:, :, 0:3], 0.0)
            else:
                nc.vector.tensor_copy(out=v[:, :, 0:3], in_=vprev[:, :, 512:515])
            nc.vector.memset(v[:, :, 515:516], 0.0)

            for j in range(4):
                yp = ypsum.tile([128, 512], f32, tag="yp")
                for k in range(4):
                    nc.tensor.matmul(
                        yp[:, :], wt_sb[:, k, 512 + 128 * j:512 + 128 * j + 128],
                        x_sb[:, k, sl], start=(k == 0), stop=(k == 3))
                cost = tpool.tile([128, 512], f32, tag="t")
                nc.scalar.activation(cost[:, :], yp[:, :], Sin,
                                     bias=bvec_sb[:, 4 + j:5 + j])
                nc.vector.tensor_mul(v[:, j, 3:515], amp_sb[:, j, sl], cost[:, :])
                sint = tpool.tile([128, 512], f32, tag="t")
                nc.scalar.activation(sint[:, :], yp[:, :], Sin,
                                     bias=bvec_sb[:, 8 + j:9 + j])
                # j == 0 row 0 (imag_0) is overwritten below by real_512
                nc.vector.tensor_mul(v[:, 4 + j, 3:515], amp_sb[:, j, sl],
                                     sint[:, :])
            # special row: v[512] = real_512 = amp512 * cos(phase512)
            c2 = spool.tile([2, 512], f32, tag="c2")
            nc.scalar.activation(c2[:, :], strip2_sb[:, sl], Sin,
                                 bias=bspec_sb[:, 1:2])
            tmp = spool.tile([1, 512], f32, tag="tmp")
            nc.sync.dma_start(out=tmp[:, :], in_=c2[1:2, :])  # partition 1 -> 0
            nc.vector.tensor_mul(v[0:1, 4, 3:515], amp512_sb[0:1, sl], tmp[:, :])

            # fused irfft + overlap-add: out_T[s, r] for 4 slot-chunks
            for q in range(4):
                ci = 4 * c + q
                op = opsum.tile([128, 256], f32, tag="op")
                idx = 0
                for j in range(4):
                    for i in range(8):
                        g = 128 * q + 3 - j
                        nc.tensor.matmul(
                            op[:, :], v[:, i, g:g + 128], bas_sb[:, 8 * j + i, :],
                            start=(idx == 0), stop=(idx == 31))
                        idx += 1
                osb = opool.tile([128, 256], f32, tag="osb")
                nc.vector.tensor_mul(osb[:, :], op[:, :], env_sb[:, ci, :])
                s0 = 128 * ci
                p0 = 2 if ci == 0 else 0   # lo half -> final[S, 128:256], S = s-2
                nc.sync.dma_start(out=out_d[s0 + p0 - 2:s0 + 126, 128:256],
                                  in_=osb[p0:128, 0:128])
                p1 = 1 if ci == 0 else 0   # hi half -> final[S, 0:128], S = s-1
                nc.sync.dma_start(out=out_d[s0 + p1 - 1:s0 + 127, 0:128],
                                  in_=osb[p1:128, 128:256])
            vprev = v

        # extra slot-chunk s0=1921 -> only slot 2048 is new
        op = opsum.tile([128, 256], f32, tag="op")
        idx = 0
        for j in range(4):
            for i in range(8):
                g = 388 - j
                nc.tensor.matmul(
                    op[:, :], vprev[:, i, g:g + 128], bas_sb[:, 8 * j + i, :],
                    start=(idx == 0), stop=(idx == 31))
                idx += 1
        osb = opool.tile([128, 256], f32, tag="osb")
        nc.vector.tensor_mul(osb[:, :], op[:, :], env_sb[:, 16, :])
        nc.sync.dma_start(out=out_d[2046:2047, 128:256], in_=osb[127:128, 0:128])
        nc.sync.dma_start(out=out_d[2047:2048, 0:128], in_=osb[127:128, 128:256])
        # zero pads at the two ends
        nc.sync.dma_start(out=out_d[0:1, 0:128], in_=ztile[:, :])
        nc.sync.dma_start(out=out_d[2047:2048, 128:256], in_=ztile[:, :])

    nc.compile()
    return nc


def kernel(x, W, b, window):
    global _compiled
    from concourse import bass_utils

    consts = _build_host_constants(W, b, window)

    if _compiled is None:
        _compiled = _build_program()
    nc = _compiled

    in_maps = []
    xf = np.asarray(x, np.float32)
    for i in range(N_CORES):
        xc = xf[i].reshape(4, 128, T).transpose(1, 0, 2)   # (128, 4, T)
        m = dict(consts)
        m["x"] = np.ascontiguousarray(xc.astype(_BF16))
        in_maps.append(m)

    res = bass_utils.run_bass_kernel_spmd(nc, in_maps, core_ids=list(range(N_CORES)))
    out = np.zeros((B, 1, OUTLEN), np.float32)
    for i in range(N_CORES):
        out[i, 0] = np.asarray(res.results[i]["out"], np.float32).reshape(-1)
    return out
